# revision 14
# baseline (speedup 1.0000x reference)
"""nn_Cvx_ShortestPathNet — TRN2 Bass kernel, 8-core pure data parallelism.

Math (Dykstra alternating projections, c folded into G via a bias lane):
    G = A' pinv(AA') A  (projector),  c = b' pinv(AA') A
    Ghat[768,768]: Ghat[:760,:760] = G, Ghat[760,:760] = -c, Ghat[760,760] = 1
    negw lane 760 := 1 (via the b2 bias constant), so t lane 760 stays 1 and
    t@Ghat == t@G - c on real lanes.
    t_1 = negw = -MLP(d);  t_{k+1} = max(negw, t_k @ Ghat)   (pure tensor max)
    y = max(negw - t_K @ Ghat, 0) = max(ps, negw) - ps

On-chip layout transposed ([768, B_local], 6x128 partition tiles), B_local =
32 per core. Per iteration: 36 fp16 matmuls (f=32 moving rows -> 25ns issue
pitch, 900ns/iter floor) + 4 DVE tensor_tensor max ops (PARTITION: two
singles + two pairs), scheduled (staggered EDF order) so each op's result is
ready close to when the next iteration's matmuls consume it. Only DVE and
Act can read PSUM; Act's ~285ns ACTIVATE is slower than DVE's 190/225ns
ops, so everything element-wise stays on DVE (and the Act engine then never
stalls its own HWDGE DMA queue — engine activity pauses that engine's
hardware queue).

PSUM bank plan: start=True clears has_written for the WHOLE bank, so two
accumulation groups may not interleave within a bank -> private banks per
group. A pair op's tile is [128,1024] fp32 = TWO banks with the groups at
the bank edges (cols 480:512 | 512:544) so its DVE max reads one contiguous
[128,64]. Ops 0,1 (consumed right at iteration start) double-buffer across
iterations via p2pool (no write-after-read wait on their banks); warm-up and
MLP matmuls use scratch columns of the fixed pair tiles (clear of 480:544).
2x2 (pairs) + 2x2 (single bufs) = all 8 banks.

Startup: each dma_start issue costs ~650ns on its engine queue and the HWDGE
ring is ~4 deep, so inputs ride NINE dmas: packed tiny tensors, three 2-chunk
W2 dmas (they gate the MLP -> negw -> t_1 chain), three 2-chunk G dmas
ordered by iteration-1 first consumption ((0,1) then (2,3) then (4,5)).
W2 is negated on the host so negw = pw + nb2c needs no Act scale=-1 pass.
Dummy warm-up matmuls keep the PE busy through the DMA phase for the HAM
clock; iteration 1 chases the G chunk arrivals.

Batch 256 sharded 32 rows per core; Ghat, MLP weights replicated.
"""

import json
import numpy as np

import concourse.bass as bass
import concourse.mybir as mybir
import concourse.tile as tile
from concourse.bass_utils import run_bass_kernel_spmd

F32 = mybir.dt.float32
F16 = mybir.dt.float16
AT = mybir.AluOpType
AF = mybir.ActivationFunctionType

JT = 6          # 768/128 edge-dim tiles
BL = 32         # batch rows per core
HT = 5          # 640/128 hidden tiles
K_ITERS = 100
N_CORES = 8
N2 = 760
NP = JT * 128
PCOL = 480      # op tile: groups end at the bank edge (480:512 | 512:544)

# DVE op partition: groups covered by each tensor_tensor max op. Singles own
# one PSUM bank; pairs own two adjacent banks (group at each bank edge).
PARTITION = [(0,), (1,), (2, 3), (4, 5)]
# Staggered (j,k) order (EDF): group closes 17/19/21/23/29/35.
MM_ORDER = [
    (0, 1), (0, 0), (1, 1), (1, 0), (0, 2), (0, 3), (1, 2), (1, 3), (2, 2),
    (2, 3), (2, 1), (2, 0), (3, 2), (3, 3), (3, 1), (3, 0), (0, 4), (0, 5),
    (1, 4), (1, 5), (2, 4), (2, 5), (3, 4), (3, 5), (4, 4), (4, 5), (4, 2),
    (4, 3), (4, 1), (4, 0), (5, 4), (5, 5), (5, 2), (5, 3), (5, 1), (5, 0),
]
L2_ORDER = (0, 1, 4, 5, 2, 3)   # MLP layer-2 j order == W2 chunk arrivals
_FIRST = {}
_LAST = {}
for _pos, (_j, _k) in enumerate(MM_ORDER):
    _FIRST.setdefault(_j, _pos)
    _LAST[_j] = _pos
_OP_OF_GROUP = {}
_IDX_IN_OP = {}
for _oi, _S in enumerate(PARTITION):
    for _ix, _g in enumerate(_S):
        _OP_OF_GROUP[_g] = _oi
        _IDX_IN_OP[_g] = _ix
P2_OPS = (0, 1)                  # double-buffered (single-group) ops


def nw_base(oi):
    return sum(len(S) for S in PARTITION[:oi]) * BL


# ---------------------------------------------------------------------------
# This container's walrus build rejects instructions carrying more than one
# sync-wait. Split any multi-wait instruction at the BIR-JSON level: insert
# same-engine NoOps before it, each carrying one of the extra waits (waits
# are sem-ge, so order is irrelevant).
_orig_to_json_bytes = bass.Bass.to_json_bytes
_ctr = [0]


def _order_waits(engine: str, waits: list) -> list:
    """NoOps take the waits that are (almost surely) already satisfied --
    same-engine sems and DMA arrivals -- so the instruction keeps the
    latest-firing cross-engine wait and pays no NoOp decode after it."""
    def prio(w):
        nm = w.get("ant_name", "")
        if nm.startswith(engine + "_"):
            return 0
        if nm.startswith("DMA"):
            return 1
        if nm.startswith("PE_"):
            return 3
        return 2
    return sorted(waits, key=prio)


def _split_waits_json(raw: bytes) -> bytes:
    j = json.loads(raw)
    changed = False
    for fn in j.get("functions", []):
        for bb in fn.get("blocks", []):
            out = []
            for inst in bb.get("instructions", []):
                si = inst.get("sync_info") or {}
                waits = si.get("on_wait") or []
                if len(waits) > 1:
                    changed = True
                    waits = _order_waits(inst.get("engine", ""), waits)
                    for w in waits[:-1]:
                        _ctr[0] += 1
                        out.append({
                            "debug": inst.get("debug", 0),
                            "engine": inst["engine"],
                            "ins": [], "outs": [],
                            "name": f"I-waitsplit-{_ctr[0]}",
                            "opcode": "NoOp",
                            "sync_info": {"on_wait": [w], "on_update": []},
                        })
                    si["on_wait"] = waits[-1:]
                out.append(inst)
            bb["instructions"] = out
    return json.dumps(j).encode() if changed else raw


def _patched_to_json_bytes(self, *a, **k):
    return _split_waits_json(_orig_to_json_bytes(self, *a, **k))


bass.Bass.to_json_bytes = _patched_to_json_bytes


def _build(k_iters=K_ITERS):
    nc = bass.Bass("TRN2", target_bir_lowering=False, debug=False,
                   num_devices=N_CORES)

    g_mat = nc.dram_tensor("g_mat", [128, JT * JT * 128], F16, kind="ExternalInput").ap()
    w2t = nc.dram_tensor("w2t", [128, HT * JT * 128], F16, kind="ExternalInput").ap()
    tiny16 = nc.dram_tensor("tiny16", [64, BL + HT * 128], F16, kind="ExternalInput").ap()
    tiny32 = nc.dram_tensor("tiny32", [128, HT + JT], F32, kind="ExternalInput").ap()
    y_out = nc.dram_tensor("y_out", [128, JT * BL], F16, kind="ExternalOutput").ap()

    NOP = len(PARTITION)

    with tile.TileContext(nc) as tc:
        with (
            tc.tile_pool(name="const", bufs=1) as cpool,
            tc.tile_pool(name="state", bufs=2) as spool,
            tc.tile_pool(name="psum", bufs=1, space="PSUM") as ppool,
            tc.tile_pool(name="psum2", bufs=2, space="PSUM") as p2pool,
        ):
            # --- input DMAs ------------------------------------------------
            t16_sb = cpool.tile([64, BL + HT * 128], F16)
            dT_sb = t16_sb[:, 0:BL]
            w1_sb = t16_sb[:, BL:BL + HT * 128]
            t32_sb = cpool.tile([128, HT + JT], F32)
            b1c_sb = t32_sb[:, 0:HT]
            nb2c_sb = t32_sb[:, HT:HT + JT]
            w2_sb = cpool.tile([128, HT * JT * 128], F16)
            G_sb = cpool.tile([128, JT * JT * 128], F16)

            def w2sl(j2):
                return slice(j2 * 2 * HT * 128, (j2 + 1) * 2 * HT * 128)

            def gsl(k2):
                return slice(k2 * 2 * JT * 128, (k2 + 1) * 2 * JT * 128)

            # Tile coarsens DMA waits toward the ring's FIFO-max dependency
            # emitted so far, so later DMA issues are EMITTED after the
            # consumers of earlier ones (per-engine issue order — and thus
            # transfer timing — is unchanged; the engines have nothing else
            # queued during the MLP).
            nc.sync.dma_start(out=t16_sb[:], in_=tiny16[:])
            nc.sync.dma_start(out=t32_sb[:], in_=tiny32[:])

            # fixed pair PSUM tiles for ops 2,3 (two banks each, groups at
            # the bank edges). Ops 0,1 get p2pool double buffers later.
            ps_fixed = {oi: ppool.tile([128, 1024], F32, tag=f"ps{oi}",
                                       name=f"psp{oi}")
                        for oi in range(NOP) if oi not in P2_OPS}
            # scratch regions for warm-up / MLP: columns clear of the pair
            # accumulation region (480:544); all scratch use completes
            # before iterations begin
            scratch = [ps_fixed[2][:, 0:BL], ps_fixed[2][:, 544:544 + BL],
                       ps_fixed[3][:, 0:BL], ps_fixed[3][:, 544:544 + BL]]
            _wctr = [0]

            def warm(n):
                # HAM warm-up: dummy matmuls keep the PE busy through the
                # DMA phase so the clock gate reaches K=8/8 before the
                # real work.
                for _ in range(n):
                    _wctr[0] += 1
                    nc.tensor.matmul(out=scratch[_wctr[0] % 4][:32, :],
                                     lhsT=dT_sb[:, :BL],
                                     rhs=dT_sb[:], start=True, stop=True)

            warm(12)

            # --- MLP (all element-wise work on DVE; Act engine stays idle
            # so its HWDGE queue streams undisturbed) -----------------------
            # h = leaky_relu(d@W1 + b1);  pw = -h@W2 (W2 negated on host);
            # negw = pw + nb2c  (nb2c = -b2, +1 on the bias lane)
            h16 = cpool.tile([128, HT * BL], F16)
            for m in range(HT):
                ph = scratch[m % 4]
                nc.tensor.matmul(out=ph[:, :],
                                 lhsT=w1_sb[:, m * 128:(m + 1) * 128],
                                 rhs=dT_sb[:], start=True, stop=True)
                pre = spool.tile([128, BL], F32, tag="pre", name=f"pre{m}")
                nc.vector.tensor_scalar(out=pre[:, :], in0=ph[:, :],
                                        scalar1=b1c_sb[:, m:m + 1],
                                        scalar2=None, op0=AT.add)
                # leaky relu = max(x, 0.1x)
                nc.vector.scalar_tensor_tensor(
                    out=h16[:, m * BL:(m + 1) * BL], in0=pre[:],
                    scalar=0.1, in1=pre[:], op0=AT.mult, op1=AT.max)

            # W2 DMA issues, emitted only now (after the L1 consumers)
            nc.gpsimd.dma_start(out=w2_sb[:, w2sl(0)], in_=w2t[:, w2sl(0)])
            nc.scalar.dma_start(out=w2_sb[:, w2sl(2)], in_=w2t[:, w2sl(2)])
            nc.sync.dma_start(out=w2_sb[:, w2sl(1)], in_=w2t[:, w2sl(1)])
            warm(40)

            negw = [cpool.tile([128, len(S) * BL], F32, name=f"negw{oi}")
                    for oi, S in enumerate(PARTITION)]
            negw16 = [cpool.tile([128, len(S) * BL], F16, name=f"negw16_{oi}")
                      for oi, S in enumerate(PARTITION)]
            for jn, j in enumerate(L2_ORDER):
                pw = scratch[(j + 1) % 4]
                for k2 in range(HT):
                    nc.tensor.matmul(
                        out=pw[:, :],
                        lhsT=w2_sb[:, (j * HT + k2) * 128:(j * HT + k2 + 1) * 128],
                        rhs=h16[:, k2 * BL:(k2 + 1) * BL],
                        start=(k2 == 0), stop=(k2 == HT - 1))
                oi, lc = _OP_OF_GROUP[j], _IDX_IN_OP[j] * BL
                nc.vector.tensor_scalar(out=negw[oi][:, lc:lc + BL],
                                        in0=pw[:, :],
                                        scalar1=nb2c_sb[:, j:j + 1],
                                        scalar2=None, op0=AT.add)
                nc.vector.tensor_copy(out=negw16[oi][:, lc:lc + BL],
                                      in_=negw[oi][:, lc:lc + BL])
                if jn % 2 == 1 and jn < JT - 1:
                    warm(10)

            # G DMA issues, emitted after the L2 consumers (iteration-1
            # matmuls then wait on exactly the G ring)
            nc.gpsimd.dma_start(out=G_sb[:, gsl(0)], in_=g_mat[:, gsl(0)])
            nc.scalar.dma_start(out=G_sb[:, gsl(1)], in_=g_mat[:, gsl(1)])
            nc.sync.dma_start(out=G_sb[:, gsl(2)], in_=g_mat[:, gsl(2)])
            # fill the PE pipe while the G wave streams in (iteration 1
            # chases the G chunk arrivals right after); the HAM clock needs
            # a ~2.5us unbroken burst here to latch the full gate count
            warm(100)

            # t_1 = negw16 op tiles directly (read-only as rhs)
            t_cur = negw16

            def rhs_of(k, tiles):
                oi = _OP_OF_GROUP[k]
                lc = _IDX_IN_OP[k] * BL
                return tiles[oi][:, lc:lc + BL]

            # --- Dykstra iterations ---------------------------------------
            for it in range(k_iters):
                last_iter = it == k_iters - 1
                ps_it = {}
                for oi in range(NOP):
                    if oi in P2_OPS:
                        ps_it[oi] = p2pool.tile([128, 512], F32,
                                                tag=f"psg{oi}",
                                                name=f"psg{oi}_{it}")
                    else:
                        ps_it[oi] = ps_fixed[oi]

                def out_ap(j):
                    oi = _OP_OF_GROUP[j]
                    c = PCOL + _IDX_IN_OP[j] * BL
                    return ps_it[oi][:, c:c + BL]

                if not last_iter:
                    t_nxt = [spool.tile([128, len(S) * BL], F16,
                                        tag=f"t{oi}", name=f"t{it + 1}_{oi}")
                             for oi, S in enumerate(PARTITION)]
                else:
                    tlf = cpool.tile([128, JT * BL], F32)
                    y_sb = cpool.tile([128, JT * BL], F16)

                left = {j: 6 for j in range(JT)}
                for pos, (j, k) in enumerate(MM_ORDER):
                    nc.tensor.matmul(
                        out=out_ap(j),
                        lhsT=G_sb[:, (k * JT + j) * 128:(k * JT + j + 1) * 128],
                        rhs=rhs_of(k, t_cur),
                        start=(pos == _FIRST[j]), stop=(pos == _LAST[j]))
                    left[j] -= 1
                    oi = _OP_OF_GROUP[j]
                    if all(left[g] == 0 for g in PARTITION[oi]):
                        for g in PARTITION[oi]:
                            left[g] = -1  # fire once
                        n = len(PARTITION[oi]) * BL
                        pss = ps_it[oi][:, PCOL:PCOL + n]
                        if not last_iter:
                            nc.vector.tensor_tensor(out=t_nxt[oi][:],
                                                    in0=pss,
                                                    in1=negw16[oi][:],
                                                    op=AT.max)
                        else:
                            gl = slice(nw_base(oi), nw_base(oi) + n)
                            nc.vector.tensor_tensor(out=tlf[:, gl], in0=pss,
                                                    in1=negw[oi][:],
                                                    op=AT.max)
                            nc.vector.tensor_tensor(out=y_sb[:, gl],
                                                    in0=tlf[:, gl], in1=pss,
                                                    op=AT.subtract)
                if not last_iter:
                    t_cur = t_nxt
                else:
                    # output DMAs fire as op results complete (ops 0+1 /
                    # op 2 / op 3), one per queue
                    nc.sync.dma_start(out=y_out[:, 0:2 * BL],
                                      in_=y_sb[:, 0:2 * BL])
                    nc.scalar.dma_start(out=y_out[:, 2 * BL:4 * BL],
                                        in_=y_sb[:, 2 * BL:4 * BL])
                    nc.gpsimd.dma_start(out=y_out[:, 4 * BL:6 * BL],
                                        in_=y_sb[:, 4 * BL:6 * BL])
    return nc


def _host_prepare(d, W1, b1, W2, b2, A, b_eq):
    A64 = A.astype(np.float64)
    M = np.linalg.pinv(A64 @ A64.T)
    G = A64.T @ M @ A64
    c = (b_eq.astype(np.float64) @ M) @ A64

    n2 = A.shape[1]
    Ghat = np.zeros((NP, NP), np.float64)
    Ghat[:n2, :n2] = G
    Ghat[n2, :n2] = -c          # bias lane row
    Ghat[n2, n2] = 1.0

    g_sb = (Ghat.reshape(JT, 128, JT, 128).transpose(1, 0, 2, 3)
            .reshape(128, JT * JT * 128)).astype(np.float16)

    HID = W1.shape[1]
    W2_pad = np.zeros((HID, NP), np.float64)
    W2_pad[:, :n2] = -W2.astype(np.float64)   # negated: negw = pw + nb2c
    w2_sb = (W2_pad.reshape(HT, 128, JT, 128).transpose(1, 2, 0, 3)
             .reshape(128, JT * HT * 128)).astype(np.float16)
    b1c = b1.reshape(HT, 128).T.astype(np.float32).copy()
    b2_pad = np.zeros(NP, np.float32)
    b2_pad[:n2] = b2
    nb2c = (-b2_pad).reshape(JT, 128).T.astype(np.float32).copy()
    nb2c[n2 - 5 * 128, 5] = 1.0   # lane 760 -> partition 120, block 5
    tiny32 = np.concatenate([b1c, nb2c], axis=1)

    shared = {"g_mat": g_sb, "w2t": w2_sb, "tiny32": tiny32}
    B = d.shape[0]
    bl = B // N_CORES
    in_maps = []
    for i in range(N_CORES):
        dT = d[i * bl:(i + 1) * bl, :].T.astype(np.float16)
        t16 = np.concatenate([dT, W1.astype(np.float16)], axis=1).copy()
        in_maps.append({**shared, "tiny16": t16})
    return in_maps


_nc_cache = {}


def kernel(d, W1, b1, W2, b2, A, b_eq):
    d = np.asarray(d, np.float32)
    W1 = np.asarray(W1, np.float32)
    b1 = np.asarray(b1, np.float32)
    W2 = np.asarray(W2, np.float32)
    b2 = np.asarray(b2, np.float32)
    A = np.asarray(A, np.float32)
    b_eq = np.asarray(b_eq, np.float32)

    if "nc" not in _nc_cache:
        _nc_cache["nc"] = _build()
    nc = _nc_cache["nc"]

    in_maps = _host_prepare(d, W1, b1, W2, b2, A, b_eq)
    res = run_bass_kernel_spmd(nc, in_maps, list(range(N_CORES)))

    outs = []
    for r in res.results:
        y = (r["y_out"].reshape(128, JT, BL).transpose(2, 1, 0)
             .reshape(BL, JT * 128))
        outs.append(y[:, :N2])
    return np.concatenate(outs, axis=0).astype(np.float32)


# revision 18
# speedup vs baseline: 1.1803x; 1.1803x over previous
"""nn_Cvx_ShortestPathNet — TRN2 Bass kernel, 8-core pure data parallelism.

Math (Dykstra alternating projections, c folded into G via a bias lane):
    G = A' pinv(AA') A  (projector),  c = b' pinv(AA') A
    Ghat[768,768]: Ghat[:760,:760] = G, Ghat[760,:760] = -c, Ghat[760,760] = 1
    negw lane 760 := 1 (via the b2 bias constant), so t lane 760 stays 1 and
    t@Ghat == t@G - c on real lanes.
    t_1 = negw = -MLP(d);  t_{k+1} = max(negw, t_k @ Ghat)   (pure tensor max)
    y = max(negw - t_K @ Ghat, 0) = max(ps, negw) - ps

On-chip layout transposed ([768, B_local], 6x128 partition tiles), B_local =
32 per core. Per iteration: 36 fp16 matmuls (f=32 moving rows -> 25ns issue
pitch, 900ns/iter floor) + 4 DVE tensor_tensor max ops (PARTITION: two
singles + two pairs), scheduled (staggered EDF order) so each op's result is
ready close to when the next iteration's matmuls consume it. Only DVE and
Act can read PSUM; Act's ~285ns ACTIVATE is slower than DVE's 190/225ns
ops, so everything element-wise stays on DVE (and the Act engine then never
stalls its own HWDGE DMA queue — engine activity pauses that engine's
hardware queue).

PSUM bank plan: start=True clears has_written for the WHOLE bank, so two
accumulation groups may not interleave within a bank -> private banks per
group. A pair op's tile is [128,1024] fp32 = TWO banks with the groups at
the bank edges (cols 480:512 | 512:544) so its DVE max reads one contiguous
[128,64]. Ops 0,1 (consumed right at iteration start) double-buffer across
iterations via p2pool (no write-after-read wait on their banks); warm-up and
MLP matmuls use scratch columns of the fixed pair tiles (clear of 480:544).
2x2 (pairs) + 2x2 (single bufs) = all 8 banks.

Startup: each dma_start issue costs ~650ns on its engine queue and the HWDGE
ring is ~4 deep, so inputs ride NINE dmas: packed tiny tensors, three 2-chunk
W2 dmas (they gate the MLP -> negw -> t_1 chain), three 2-chunk G dmas
ordered by iteration-1 first consumption ((0,1) then (2,3) then (4,5)).
W2 is negated on the host so negw = pw + nb2c needs no Act scale=-1 pass.
Dummy warm-up matmuls keep the PE busy through the DMA phase for the HAM
clock; iteration 1 chases the G chunk arrivals.

Batch 256 sharded 32 rows per core; Ghat, MLP weights replicated.
"""

import json
import numpy as np

import concourse.bass as bass
import concourse.mybir as mybir
import concourse.tile as tile
from concourse.bass_utils import run_bass_kernel_spmd

F32 = mybir.dt.float32
F16 = mybir.dt.float16
AT = mybir.AluOpType
AF = mybir.ActivationFunctionType

JT = 6          # 768/128 edge-dim tiles
BL = 32         # batch rows per core
HT = 5          # 640/128 hidden tiles
K_ITERS = 100
N_CORES = 8
N2 = 760
NP = JT * 128
PCOL = 480      # op tile: groups end at the bank edge (480:512 | 512:544)

# DVE op partition: groups covered by each tensor_tensor max op. Singles own
# one PSUM bank; pairs own two adjacent banks (group at each bank edge).
PARTITION = [(0,), (1,), (2, 3), (4, 5)]
# Staggered (j,k) order (EDF): group closes 17/19/21/23/29/35.
MM_ORDER = [
    (0, 1), (0, 0), (1, 1), (1, 0), (0, 2), (0, 3), (1, 2), (1, 3), (2, 2),
    (2, 3), (2, 1), (2, 0), (3, 2), (3, 3), (3, 1), (3, 0), (0, 4), (0, 5),
    (1, 4), (1, 5), (2, 4), (2, 5), (3, 4), (3, 5), (4, 4), (4, 5), (4, 2),
    (4, 3), (4, 1), (4, 0), (5, 4), (5, 5), (5, 2), (5, 3), (5, 1), (5, 0),
]
L2_ORDER = (2, 3, 4, 5, 0, 1)   # MLP layer-2 j order == W2 chunk arrivals
_FIRST = {}
_LAST = {}
for _pos, (_j, _k) in enumerate(MM_ORDER):
    _FIRST.setdefault(_j, _pos)
    _LAST[_j] = _pos
_OP_OF_GROUP = {}
_IDX_IN_OP = {}
for _oi, _S in enumerate(PARTITION):
    for _ix, _g in enumerate(_S):
        _OP_OF_GROUP[_g] = _oi
        _IDX_IN_OP[_g] = _ix
P2_OPS = (0, 1)                  # double-buffered (single-group) ops


def nw_base(oi):
    return sum(len(S) for S in PARTITION[:oi]) * BL


# ---------------------------------------------------------------------------
# This container's walrus build rejects instructions carrying more than one
# sync-wait. Split any multi-wait instruction at the BIR-JSON level: insert
# same-engine NoOps before it, each carrying one of the extra waits (waits
# are sem-ge, so order is irrelevant).
_orig_to_json_bytes = bass.Bass.to_json_bytes
_ctr = [0]


def _order_waits(engine: str, waits: list) -> list:
    """NoOps take the waits that are (almost surely) already satisfied --
    same-engine sems and DMA arrivals -- so the instruction keeps the
    latest-firing cross-engine wait and pays no NoOp decode after it."""
    def prio(w):
        nm = w.get("ant_name", "")
        if nm.startswith(engine + "_"):
            return 0
        if nm.startswith("DMA"):
            return 1
        if nm.startswith("PE_"):
            return 3
        return 2
    return sorted(waits, key=prio)


def _split_waits_json(raw: bytes) -> bytes:
    j = json.loads(raw)
    changed = False
    for fn in j.get("functions", []):
        for bb in fn.get("blocks", []):
            out = []
            for inst in bb.get("instructions", []):
                si = inst.get("sync_info") or {}
                waits = si.get("on_wait") or []
                if len(waits) > 1:
                    changed = True
                    waits = _order_waits(inst.get("engine", ""), waits)
                    for w in waits[:-1]:
                        _ctr[0] += 1
                        out.append({
                            "debug": inst.get("debug", 0),
                            "engine": inst["engine"],
                            "ins": [], "outs": [],
                            "name": f"I-waitsplit-{_ctr[0]}",
                            "opcode": "NoOp",
                            "sync_info": {"on_wait": [w], "on_update": []},
                        })
                    si["on_wait"] = waits[-1:]
                out.append(inst)
            bb["instructions"] = out
    return json.dumps(j).encode() if changed else raw


def _patched_to_json_bytes(self, *a, **k):
    return _split_waits_json(_orig_to_json_bytes(self, *a, **k))


bass.Bass.to_json_bytes = _patched_to_json_bytes


def _build(k_iters=K_ITERS):
    nc = bass.Bass("TRN2", target_bir_lowering=False, debug=False,
                   num_devices=N_CORES)

    g_mat = nc.dram_tensor("g_mat", [128, JT * JT * 128], F16, kind="ExternalInput").ap()
    w2t = nc.dram_tensor("w2t", [128, HT * JT * 128], F16, kind="ExternalInput").ap()
    tiny16 = nc.dram_tensor("tiny16", [64, BL + HT * 128], F16, kind="ExternalInput").ap()
    tiny32 = nc.dram_tensor("tiny32", [128, HT + JT], F32, kind="ExternalInput").ap()
    y_out = nc.dram_tensor("y_out", [128, JT * BL], F16, kind="ExternalOutput").ap()

    NOP = len(PARTITION)

    with tile.TileContext(nc) as tc:
        with (
            tc.tile_pool(name="const", bufs=1) as cpool,
            tc.tile_pool(name="state", bufs=2) as spool,
            tc.tile_pool(name="psum", bufs=1, space="PSUM") as ppool,
            tc.tile_pool(name="psum2", bufs=2, space="PSUM") as p2pool,
        ):
            # --- input DMAs ------------------------------------------------
            t16_sb = cpool.tile([64, BL + HT * 128], F16)
            dT_sb = t16_sb[:, 0:BL]
            w1_sb = t16_sb[:, BL:BL + HT * 128]
            t32_sb = cpool.tile([128, HT + JT], F32)
            b1c_sb = t32_sb[:, 0:HT]
            nb2c_sb = t32_sb[:, HT:HT + JT]
            w2_sb = cpool.tile([128, HT * JT * 128], F16)
            G_sb = cpool.tile([128, JT * JT * 128], F16)

            def w2sl(j2):
                return slice(j2 * 2 * HT * 128, (j2 + 1) * 2 * HT * 128)

            def gsl(k2):
                return slice(k2 * 2 * JT * 128, (k2 + 1) * 2 * JT * 128)

            # All DMAs issue up front (mid-program dma emission sprinkles
            # satisfied waits over downstream instructions, +5ns per matmul).
            # The MLP-critical tensors ride SWDGE (gpsimd), whose DMAs carry
            # INDIVIDUAL completion sems — HWDGE ring waits coarsen toward
            # the ring's FIFO-max dependency, so sync/scalar only carry
            # data whose consumers run late anyway.
            nc.gpsimd.dma_start(out=t16_sb[:], in_=tiny16[:])
            nc.gpsimd.dma_start(out=t32_sb[:], in_=tiny32[:])
            nc.gpsimd.dma_start(out=w2_sb[:, w2sl(0)], in_=w2t[:, w2sl(0)])
            nc.sync.dma_start(out=w2_sb[:, w2sl(1)], in_=w2t[:, w2sl(1)])
            nc.scalar.dma_start(out=w2_sb[:, w2sl(2)], in_=w2t[:, w2sl(2)])
            nc.gpsimd.dma_start(out=G_sb[:, gsl(0)], in_=g_mat[:, gsl(0)])
            nc.sync.dma_start(out=G_sb[:, gsl(2)], in_=g_mat[:, gsl(2)])
            nc.scalar.dma_start(out=G_sb[:, gsl(1)], in_=g_mat[:, gsl(1)])

            # fixed pair PSUM tiles for ops 2,3 (two banks each, groups at
            # the bank edges). Ops 0,1 get p2pool double buffers later.
            ps_fixed = {oi: ppool.tile([128, 1024], F32, tag=f"ps{oi}",
                                       name=f"psp{oi}")
                        for oi in range(NOP) if oi not in P2_OPS}
            # scratch regions for warm-up / MLP: columns clear of the pair
            # accumulation region (480:544); all scratch use completes
            # before iterations begin
            scratch = [ps_fixed[2][:, 0:BL], ps_fixed[2][:, 544:544 + BL],
                       ps_fixed[3][:, 0:BL], ps_fixed[3][:, 544:544 + BL]]
            _wctr = [0]

            def warm(n):
                # HAM warm-up: dummy matmuls keep the PE busy through the
                # DMA phase so the clock gate reaches K=8/8 before the
                # real work.
                for _ in range(n):
                    _wctr[0] += 1
                    nc.tensor.matmul(out=scratch[_wctr[0] % 4][:32, :],
                                     lhsT=dT_sb[:, :BL],
                                     rhs=dT_sb[:], start=True, stop=True)

            warm(12)

            # --- MLP (all element-wise work on DVE; Act engine stays idle
            # so its HWDGE queue streams undisturbed) -----------------------
            # h = leaky_relu(d@W1 + b1);  pw = -h@W2 (W2 negated on host);
            # negw = pw + nb2c  (nb2c = -b2, +1 on the bias lane)
            h16 = cpool.tile([128, HT * BL], F16)
            for m in range(HT):
                ph = scratch[m % 4]
                nc.tensor.matmul(out=ph[:, :],
                                 lhsT=w1_sb[:, m * 128:(m + 1) * 128],
                                 rhs=dT_sb[:], start=True, stop=True)
                pre = spool.tile([128, BL], F32, tag="pre", name=f"pre{m}")
                nc.vector.tensor_scalar(out=pre[:, :], in0=ph[:, :],
                                        scalar1=b1c_sb[:, m:m + 1],
                                        scalar2=None, op0=AT.add)
                # leaky relu = max(x, 0.1x)
                nc.vector.scalar_tensor_tensor(
                    out=h16[:, m * BL:(m + 1) * BL], in0=pre[:],
                    scalar=0.1, in1=pre[:], op0=AT.mult, op1=AT.max)
            warm(40)

            negw = [cpool.tile([128, len(S) * BL], F32, name=f"negw{oi}")
                    for oi, S in enumerate(PARTITION)]
            negw16 = [cpool.tile([128, len(S) * BL], F16, name=f"negw16_{oi}")
                      for oi, S in enumerate(PARTITION)]
            for jn, j in enumerate(L2_ORDER):
                pw = scratch[(j + 1) % 4]
                for k2 in range(HT):
                    nc.tensor.matmul(
                        out=pw[:, :],
                        lhsT=w2_sb[:, (j * HT + k2) * 128:(j * HT + k2 + 1) * 128],
                        rhs=h16[:, k2 * BL:(k2 + 1) * BL],
                        start=(k2 == 0), stop=(k2 == HT - 1))
                oi, lc = _OP_OF_GROUP[j], _IDX_IN_OP[j] * BL
                nc.vector.tensor_scalar(out=negw[oi][:, lc:lc + BL],
                                        in0=pw[:, :],
                                        scalar1=nb2c_sb[:, j:j + 1],
                                        scalar2=None, op0=AT.add)
                nc.vector.tensor_copy(out=negw16[oi][:, lc:lc + BL],
                                      in_=negw[oi][:, lc:lc + BL])
                if jn % 2 == 1 and jn < JT - 1:
                    warm(10)
            # fill the PE pipe while the G wave streams in (iteration 1
            # chases the G chunk arrivals right after); the HAM clock needs
            # a ~2.5us unbroken burst here to latch the full gate count
            warm(100)

            # t_1 = negw16 op tiles directly (read-only as rhs)
            t_cur = negw16

            def rhs_of(k, tiles):
                oi = _OP_OF_GROUP[k]
                lc = _IDX_IN_OP[k] * BL
                return tiles[oi][:, lc:lc + BL]

            # --- Dykstra iterations ---------------------------------------
            for it in range(k_iters):
                last_iter = it == k_iters - 1
                ps_it = {}
                for oi in range(NOP):
                    if oi in P2_OPS:
                        ps_it[oi] = p2pool.tile([128, 512], F32,
                                                tag=f"psg{oi}",
                                                name=f"psg{oi}_{it}")
                    else:
                        ps_it[oi] = ps_fixed[oi]

                def out_ap(j):
                    oi = _OP_OF_GROUP[j]
                    c = PCOL + _IDX_IN_OP[j] * BL
                    return ps_it[oi][:, c:c + BL]

                if not last_iter:
                    t_nxt = [spool.tile([128, len(S) * BL], F16,
                                        tag=f"t{oi}", name=f"t{it + 1}_{oi}")
                             for oi, S in enumerate(PARTITION)]
                else:
                    tlf = cpool.tile([128, JT * BL], F32)
                    y_sb = cpool.tile([128, JT * BL], F16)

                left = {j: 6 for j in range(JT)}
                for pos, (j, k) in enumerate(MM_ORDER):
                    nc.tensor.matmul(
                        out=out_ap(j),
                        lhsT=G_sb[:, (k * JT + j) * 128:(k * JT + j + 1) * 128],
                        rhs=rhs_of(k, t_cur),
                        start=(pos == _FIRST[j]), stop=(pos == _LAST[j]))
                    left[j] -= 1
                    oi = _OP_OF_GROUP[j]
                    if all(left[g] == 0 for g in PARTITION[oi]):
                        for g in PARTITION[oi]:
                            left[g] = -1  # fire once
                        n = len(PARTITION[oi]) * BL
                        pss = ps_it[oi][:, PCOL:PCOL + n]
                        if not last_iter:
                            nc.vector.tensor_tensor(out=t_nxt[oi][:],
                                                    in0=pss,
                                                    in1=negw16[oi][:],
                                                    op=AT.max)
                        else:
                            gl = slice(nw_base(oi), nw_base(oi) + n)
                            nc.vector.tensor_tensor(out=tlf[:, gl], in0=pss,
                                                    in1=negw[oi][:],
                                                    op=AT.max)
                            nc.vector.tensor_tensor(out=y_sb[:, gl],
                                                    in0=tlf[:, gl], in1=pss,
                                                    op=AT.subtract)
                if not last_iter:
                    t_cur = t_nxt
                else:
                    # output DMAs fire as op results complete (ops 0+1 /
                    # op 2 / op 3), one per queue
                    nc.sync.dma_start(out=y_out[:, 0:2 * BL],
                                      in_=y_sb[:, 0:2 * BL])
                    nc.scalar.dma_start(out=y_out[:, 2 * BL:4 * BL],
                                        in_=y_sb[:, 2 * BL:4 * BL])
                    nc.gpsimd.dma_start(out=y_out[:, 4 * BL:6 * BL],
                                        in_=y_sb[:, 4 * BL:6 * BL])
    return nc


def _host_prepare(d, W1, b1, W2, b2, A, b_eq):
    A64 = A.astype(np.float64)
    M = np.linalg.pinv(A64 @ A64.T)
    G = A64.T @ M @ A64
    c = (b_eq.astype(np.float64) @ M) @ A64

    n2 = A.shape[1]
    Ghat = np.zeros((NP, NP), np.float64)
    Ghat[:n2, :n2] = G
    Ghat[n2, :n2] = -c          # bias lane row
    Ghat[n2, n2] = 1.0

    g_sb = (Ghat.reshape(JT, 128, JT, 128).transpose(1, 0, 2, 3)
            .reshape(128, JT * JT * 128)).astype(np.float16)

    HID = W1.shape[1]
    W2_pad = np.zeros((HID, NP), np.float64)
    W2_pad[:, :n2] = -W2.astype(np.float64)   # negated: negw = pw + nb2c
    w2_sb = (W2_pad.reshape(HT, 128, JT, 128).transpose(1, 2, 0, 3)
             .reshape(128, JT * HT * 128)).astype(np.float16)
    b1c = b1.reshape(HT, 128).T.astype(np.float32).copy()
    b2_pad = np.zeros(NP, np.float32)
    b2_pad[:n2] = b2
    nb2c = (-b2_pad).reshape(JT, 128).T.astype(np.float32).copy()
    nb2c[n2 - 5 * 128, 5] = 1.0   # lane 760 -> partition 120, block 5
    tiny32 = np.concatenate([b1c, nb2c], axis=1)

    shared = {"g_mat": g_sb, "w2t": w2_sb, "tiny32": tiny32}
    B = d.shape[0]
    bl = B // N_CORES
    in_maps = []
    for i in range(N_CORES):
        dT = d[i * bl:(i + 1) * bl, :].T.astype(np.float16)
        t16 = np.concatenate([dT, W1.astype(np.float16)], axis=1).copy()
        in_maps.append({**shared, "tiny16": t16})
    return in_maps


_nc_cache = {}


def kernel(d, W1, b1, W2, b2, A, b_eq):
    d = np.asarray(d, np.float32)
    W1 = np.asarray(W1, np.float32)
    b1 = np.asarray(b1, np.float32)
    W2 = np.asarray(W2, np.float32)
    b2 = np.asarray(b2, np.float32)
    A = np.asarray(A, np.float32)
    b_eq = np.asarray(b_eq, np.float32)

    if "nc" not in _nc_cache:
        _nc_cache["nc"] = _build()
    nc = _nc_cache["nc"]

    in_maps = _host_prepare(d, W1, b1, W2, b2, A, b_eq)
    res = run_bass_kernel_spmd(nc, in_maps, list(range(N_CORES)))

    outs = []
    for r in res.results:
        y = (r["y_out"].reshape(128, JT, BL).transpose(2, 1, 0)
             .reshape(BL, JT * 128))
        outs.append(y[:, :N2])
    return np.concatenate(outs, axis=0).astype(np.float32)


# revision 20
# speedup vs baseline: 1.1962x; 1.0134x over previous
"""nn_Cvx_ShortestPathNet — TRN2 Bass kernel, 8-core pure data parallelism.

Math (Dykstra alternating projections, c folded into G via a bias lane):
    G = A' pinv(AA') A  (projector),  c = b' pinv(AA') A
    Ghat[768,768]: Ghat[:760,:760] = G, Ghat[760,:760] = -c, Ghat[760,760] = 1
    negw lane 760 := 1 (via the b2 bias constant), so t lane 760 stays 1 and
    t@Ghat == t@G - c on real lanes.
    t_1 = negw = -MLP(d);  t_{k+1} = max(negw, t_k @ Ghat)   (pure tensor max)
    y = max(negw - t_K @ Ghat, 0) = max(ps, negw) - ps

On-chip layout transposed ([768, B_local], 6x128 partition tiles), B_local =
32 per core. Per iteration: 36 fp16 matmuls (f=32 moving rows -> 25ns issue
pitch, 900ns/iter floor) + 4 DVE tensor_tensor max ops (PARTITION: two
singles + two pairs), scheduled (staggered EDF order) so each op's result is
ready close to when the next iteration's matmuls consume it. Only DVE and
Act can read PSUM; Act's ~285ns ACTIVATE is slower than DVE's 190/225ns
ops, so everything element-wise stays on DVE (and the Act engine then never
stalls its own HWDGE DMA queue — engine activity pauses that engine's
hardware queue).

PSUM bank plan: start=True clears has_written for the WHOLE bank, so two
accumulation groups may not interleave within a bank -> private banks per
group. A pair op's tile is [128,1024] fp32 = TWO banks with the groups at
the bank edges (cols 480:512 | 512:544) so its DVE max reads one contiguous
[128,64]. Ops 0,1 (consumed right at iteration start) double-buffer across
iterations via p2pool (no write-after-read wait on their banks); warm-up and
MLP matmuls use scratch columns of the fixed pair tiles (clear of 480:544).
2x2 (pairs) + 2x2 (single bufs) = all 8 banks.

Startup: each dma_start issue costs ~650ns on its engine queue and the HWDGE
ring is ~4 deep, so inputs ride NINE dmas: packed tiny tensors, three 2-chunk
W2 dmas (they gate the MLP -> negw -> t_1 chain), three 2-chunk G dmas
ordered by iteration-1 first consumption ((0,1) then (2,3) then (4,5)).
W2 is negated on the host so negw = pw + nb2c needs no Act scale=-1 pass.
Dummy warm-up matmuls keep the PE busy through the DMA phase for the HAM
clock; iteration 1 chases the G chunk arrivals.

Batch 256 sharded 32 rows per core; Ghat, MLP weights replicated.
"""

import json
import numpy as np

import concourse.bass as bass
import concourse.mybir as mybir
import concourse.tile as tile
from concourse.bass_utils import run_bass_kernel_spmd

F32 = mybir.dt.float32
F16 = mybir.dt.float16
AT = mybir.AluOpType
AF = mybir.ActivationFunctionType

JT = 6          # 768/128 edge-dim tiles
BL = 32         # batch rows per core
HT = 5          # 640/128 hidden tiles
K_ITERS = 100
N_CORES = 8
N2 = 760
NP = JT * 128
PCOL = 480      # op tile: groups end at the bank edge (480:512 | 512:544)

# DVE op partition: groups covered by each tensor_tensor max op. Singles own
# one PSUM bank; pairs own two adjacent banks (group at each bank edge).
PARTITION = [(0,), (1,), (2, 3), (4, 5)]
# Staggered (j,k) order (EDF): group closes 17/19/21/23/29/35.
MM_ORDER = [
    (0, 1), (0, 0), (1, 1), (1, 0), (0, 2), (0, 3), (1, 2), (1, 3), (2, 2),
    (2, 3), (2, 1), (2, 0), (3, 2), (3, 3), (3, 1), (3, 0), (0, 4), (0, 5),
    (1, 4), (1, 5), (2, 4), (2, 5), (3, 4), (3, 5), (4, 4), (4, 5), (4, 2),
    (4, 3), (4, 1), (4, 0), (5, 4), (5, 5), (5, 2), (5, 3), (5, 1), (5, 0),
]
L2_ORDER = (4, 5, 2, 3, 0, 1)   # MLP layer-2 j order == W2 chunk arrivals
_FIRST = {}
_LAST = {}
for _pos, (_j, _k) in enumerate(MM_ORDER):
    _FIRST.setdefault(_j, _pos)
    _LAST[_j] = _pos
_OP_OF_GROUP = {}
_IDX_IN_OP = {}
for _oi, _S in enumerate(PARTITION):
    for _ix, _g in enumerate(_S):
        _OP_OF_GROUP[_g] = _oi
        _IDX_IN_OP[_g] = _ix
P2_OPS = (0, 1)                  # double-buffered (single-group) ops


def nw_base(oi):
    return sum(len(S) for S in PARTITION[:oi]) * BL


# ---------------------------------------------------------------------------
# This container's walrus build rejects instructions carrying more than one
# sync-wait. Split any multi-wait instruction at the BIR-JSON level: insert
# same-engine NoOps before it, each carrying one of the extra waits (waits
# are sem-ge, so order is irrelevant).
_orig_to_json_bytes = bass.Bass.to_json_bytes
_ctr = [0]


def _order_waits(engine: str, waits: list) -> list:
    """NoOps take the waits that are (almost surely) already satisfied --
    same-engine sems and DMA arrivals -- so the instruction keeps the
    latest-firing cross-engine wait and pays no NoOp decode after it."""
    def prio(w):
        nm = w.get("ant_name", "")
        if nm.startswith(engine + "_"):
            return 0
        if nm.startswith("DMA"):
            return 1
        if nm.startswith("PE_"):
            return 3
        return 2
    return sorted(waits, key=prio)


def _split_waits_json(raw: bytes) -> bytes:
    j = json.loads(raw)
    changed = False
    for fn in j.get("functions", []):
        for bb in fn.get("blocks", []):
            out = []
            for inst in bb.get("instructions", []):
                si = inst.get("sync_info") or {}
                waits = si.get("on_wait") or []
                if len(waits) > 1:
                    changed = True
                    waits = _order_waits(inst.get("engine", ""), waits)
                    for w in waits[:-1]:
                        _ctr[0] += 1
                        out.append({
                            "debug": inst.get("debug", 0),
                            "engine": inst["engine"],
                            "ins": [], "outs": [],
                            "name": f"I-waitsplit-{_ctr[0]}",
                            "opcode": "NoOp",
                            "sync_info": {"on_wait": [w], "on_update": []},
                        })
                    si["on_wait"] = waits[-1:]
                out.append(inst)
            bb["instructions"] = out
    return json.dumps(j).encode() if changed else raw


def _patched_to_json_bytes(self, *a, **k):
    return _split_waits_json(_orig_to_json_bytes(self, *a, **k))


bass.Bass.to_json_bytes = _patched_to_json_bytes


def _build(k_iters=K_ITERS):
    nc = bass.Bass("TRN2", target_bir_lowering=False, debug=False,
                   num_devices=N_CORES)

    g_mat = nc.dram_tensor("g_mat", [128, JT * JT * 128], F16, kind="ExternalInput").ap()
    w2t = nc.dram_tensor("w2t", [128, HT * JT * 128], F16, kind="ExternalInput").ap()
    tiny16 = nc.dram_tensor("tiny16", [64, BL + HT * 128], F16, kind="ExternalInput").ap()
    tiny32 = nc.dram_tensor("tiny32", [128, HT + JT], F32, kind="ExternalInput").ap()
    y_out = nc.dram_tensor("y_out", [128, JT * BL], F16, kind="ExternalOutput").ap()

    NOP = len(PARTITION)

    with tile.TileContext(nc) as tc:
        with (
            tc.tile_pool(name="const", bufs=1) as cpool,
            tc.tile_pool(name="state", bufs=2) as spool,
            tc.tile_pool(name="psum", bufs=1, space="PSUM") as ppool,
            tc.tile_pool(name="psum2", bufs=2, space="PSUM") as p2pool,
        ):
            # --- input DMAs ------------------------------------------------
            t16_sb = cpool.tile([64, BL + HT * 128], F16)
            dT_sb = t16_sb[:, 0:BL]
            w1_sb = t16_sb[:, BL:BL + HT * 128]
            t32_sb = cpool.tile([128, HT + JT], F32)
            b1c_sb = t32_sb[:, 0:HT]
            nb2c_sb = t32_sb[:, HT:HT + JT]
            w2_sb = cpool.tile([128, HT * JT * 128], F16)
            G_sb = cpool.tile([128, JT * JT * 128], F16)

            def w2sl(j2):
                return slice(j2 * 2 * HT * 128, (j2 + 1) * 2 * HT * 128)

            def gsl(k2):
                return slice(k2 * 2 * JT * 128, (k2 + 1) * 2 * JT * 128)

            # All DMAs issue up front (mid-program dma emission sprinkles
            # satisfied waits over downstream instructions, +5ns per matmul).
            # Empirically a consumer of DMA #k on a queue is released only
            # when DMA #k+1 on that queue completes, so each early-needed
            # DMA is chased by a 1KB dummy; the G chunks sit last per queue.
            dmy = [cpool.tile([128, 1], F32, name=f"dmy{i}") for i in range(4)]

            nc.sync.dma_start(out=t16_sb[:], in_=tiny16[:])
            nc.sync.dma_start(out=t32_sb[:], in_=tiny32[:])
            nc.sync.dma_start(out=dmy[0][:], in_=tiny32[:, 0:1])
            nc.sync.dma_start(out=w2_sb[:, w2sl(1)], in_=w2t[:, w2sl(1)])
            nc.sync.dma_start(out=dmy[1][:], in_=tiny32[:, 1:2])
            nc.sync.dma_start(out=G_sb[:, gsl(1)], in_=g_mat[:, gsl(1)])

            nc.scalar.dma_start(out=w2_sb[:, w2sl(2)], in_=w2t[:, w2sl(2)])
            nc.scalar.dma_start(out=dmy[2][:], in_=tiny32[:, 2:3])
            nc.scalar.dma_start(out=G_sb[:, gsl(0)], in_=g_mat[:, gsl(0)])

            nc.gpsimd.dma_start(out=w2_sb[:, w2sl(0)], in_=w2t[:, w2sl(0)])
            nc.gpsimd.dma_start(out=dmy[3][:], in_=tiny32[:, 3:4])
            nc.gpsimd.dma_start(out=G_sb[:, gsl(2)], in_=g_mat[:, gsl(2)])

            # fixed pair PSUM tiles for ops 2,3 (two banks each, groups at
            # the bank edges). Ops 0,1 get p2pool double buffers later.
            ps_fixed = {oi: ppool.tile([128, 1024], F32, tag=f"ps{oi}",
                                       name=f"psp{oi}")
                        for oi in range(NOP) if oi not in P2_OPS}
            # scratch regions for warm-up / MLP: columns clear of the pair
            # accumulation region (480:544); all scratch use completes
            # before iterations begin
            scratch = [ps_fixed[2][:, 0:BL], ps_fixed[2][:, 544:544 + BL],
                       ps_fixed[3][:, 0:BL], ps_fixed[3][:, 544:544 + BL]]
            _wctr = [0]

            def warm(n):
                # HAM warm-up: dummy matmuls keep the PE busy through the
                # DMA phase so the clock gate reaches K=8/8 before the
                # real work.
                for _ in range(n):
                    _wctr[0] += 1
                    nc.tensor.matmul(out=scratch[_wctr[0] % 4][:32, :],
                                     lhsT=dT_sb[:, :BL],
                                     rhs=dT_sb[:], start=True, stop=True)

            warm(12)

            # --- MLP (all element-wise work on DVE; Act engine stays idle
            # so its HWDGE queue streams undisturbed) -----------------------
            # h = leaky_relu(d@W1 + b1);  pw = -h@W2 (W2 negated on host);
            # negw = pw + nb2c  (nb2c = -b2, +1 on the bias lane)
            h16 = cpool.tile([128, HT * BL], F16)
            for m in range(HT):
                ph = scratch[m % 4]
                nc.tensor.matmul(out=ph[:, :],
                                 lhsT=w1_sb[:, m * 128:(m + 1) * 128],
                                 rhs=dT_sb[:], start=True, stop=True)
                pre = spool.tile([128, BL], F32, tag="pre", name=f"pre{m}")
                nc.vector.tensor_scalar(out=pre[:, :], in0=ph[:, :],
                                        scalar1=b1c_sb[:, m:m + 1],
                                        scalar2=None, op0=AT.add)
                # leaky relu = max(x, 0.1x)
                nc.vector.scalar_tensor_tensor(
                    out=h16[:, m * BL:(m + 1) * BL], in0=pre[:],
                    scalar=0.1, in1=pre[:], op0=AT.mult, op1=AT.max)
            warm(40)

            negw = [cpool.tile([128, len(S) * BL], F32, name=f"negw{oi}")
                    for oi, S in enumerate(PARTITION)]
            negw16 = [cpool.tile([128, len(S) * BL], F16, name=f"negw16_{oi}")
                      for oi, S in enumerate(PARTITION)]
            for jn, j in enumerate(L2_ORDER):
                pw = scratch[(j + 1) % 4]
                for k2 in range(HT):
                    nc.tensor.matmul(
                        out=pw[:, :],
                        lhsT=w2_sb[:, (j * HT + k2) * 128:(j * HT + k2 + 1) * 128],
                        rhs=h16[:, k2 * BL:(k2 + 1) * BL],
                        start=(k2 == 0), stop=(k2 == HT - 1))
                oi, lc = _OP_OF_GROUP[j], _IDX_IN_OP[j] * BL
                nc.vector.tensor_scalar(out=negw[oi][:, lc:lc + BL],
                                        in0=pw[:, :],
                                        scalar1=nb2c_sb[:, j:j + 1],
                                        scalar2=None, op0=AT.add)
                nc.vector.tensor_copy(out=negw16[oi][:, lc:lc + BL],
                                      in_=negw[oi][:, lc:lc + BL])
                if jn % 2 == 1 and jn < JT - 1:
                    warm(10)
            # fill the PE pipe while the G wave streams in (iteration 1
            # chases the G chunk arrivals right after); the HAM clock needs
            # a ~2.5us unbroken burst here to latch the full gate count
            warm(100)

            # t_1 = negw16 op tiles directly (read-only as rhs)
            t_cur = negw16

            def rhs_of(k, tiles):
                oi = _OP_OF_GROUP[k]
                lc = _IDX_IN_OP[k] * BL
                return tiles[oi][:, lc:lc + BL]

            # --- Dykstra iterations ---------------------------------------
            for it in range(k_iters):
                last_iter = it == k_iters - 1
                ps_it = {}
                for oi in range(NOP):
                    if oi in P2_OPS:
                        ps_it[oi] = p2pool.tile([128, 512], F32,
                                                tag=f"psg{oi}",
                                                name=f"psg{oi}_{it}")
                    else:
                        ps_it[oi] = ps_fixed[oi]

                def out_ap(j):
                    oi = _OP_OF_GROUP[j]
                    c = PCOL + _IDX_IN_OP[j] * BL
                    return ps_it[oi][:, c:c + BL]

                if not last_iter:
                    t_nxt = [spool.tile([128, len(S) * BL], F16,
                                        tag=f"t{oi}", name=f"t{it + 1}_{oi}")
                             for oi, S in enumerate(PARTITION)]
                else:
                    tlf = cpool.tile([128, JT * BL], F32)
                    y_sb = cpool.tile([128, JT * BL], F16)

                left = {j: 6 for j in range(JT)}
                for pos, (j, k) in enumerate(MM_ORDER):
                    nc.tensor.matmul(
                        out=out_ap(j),
                        lhsT=G_sb[:, (k * JT + j) * 128:(k * JT + j + 1) * 128],
                        rhs=rhs_of(k, t_cur),
                        start=(pos == _FIRST[j]), stop=(pos == _LAST[j]))
                    left[j] -= 1
                    oi = _OP_OF_GROUP[j]
                    if all(left[g] == 0 for g in PARTITION[oi]):
                        for g in PARTITION[oi]:
                            left[g] = -1  # fire once
                        n = len(PARTITION[oi]) * BL
                        pss = ps_it[oi][:, PCOL:PCOL + n]
                        if not last_iter:
                            nc.vector.tensor_tensor(out=t_nxt[oi][:],
                                                    in0=pss,
                                                    in1=negw16[oi][:],
                                                    op=AT.max)
                        else:
                            gl = slice(nw_base(oi), nw_base(oi) + n)
                            nc.vector.tensor_tensor(out=tlf[:, gl], in0=pss,
                                                    in1=negw[oi][:],
                                                    op=AT.max)
                            nc.vector.tensor_tensor(out=y_sb[:, gl],
                                                    in0=tlf[:, gl], in1=pss,
                                                    op=AT.subtract)
                if not last_iter:
                    t_cur = t_nxt
                else:
                    # output DMAs fire as op results complete (ops 0+1 /
                    # op 2 / op 3), one per queue
                    nc.sync.dma_start(out=y_out[:, 0:2 * BL],
                                      in_=y_sb[:, 0:2 * BL])
                    nc.scalar.dma_start(out=y_out[:, 2 * BL:4 * BL],
                                        in_=y_sb[:, 2 * BL:4 * BL])
                    nc.gpsimd.dma_start(out=y_out[:, 4 * BL:6 * BL],
                                        in_=y_sb[:, 4 * BL:6 * BL])
    return nc


def _host_prepare(d, W1, b1, W2, b2, A, b_eq):
    A64 = A.astype(np.float64)
    M = np.linalg.pinv(A64 @ A64.T)
    G = A64.T @ M @ A64
    c = (b_eq.astype(np.float64) @ M) @ A64

    n2 = A.shape[1]
    Ghat = np.zeros((NP, NP), np.float64)
    Ghat[:n2, :n2] = G
    Ghat[n2, :n2] = -c          # bias lane row
    Ghat[n2, n2] = 1.0

    g_sb = (Ghat.reshape(JT, 128, JT, 128).transpose(1, 0, 2, 3)
            .reshape(128, JT * JT * 128)).astype(np.float16)

    HID = W1.shape[1]
    W2_pad = np.zeros((HID, NP), np.float64)
    W2_pad[:, :n2] = -W2.astype(np.float64)   # negated: negw = pw + nb2c
    w2_sb = (W2_pad.reshape(HT, 128, JT, 128).transpose(1, 2, 0, 3)
             .reshape(128, JT * HT * 128)).astype(np.float16)
    b1c = b1.reshape(HT, 128).T.astype(np.float32).copy()
    b2_pad = np.zeros(NP, np.float32)
    b2_pad[:n2] = b2
    nb2c = (-b2_pad).reshape(JT, 128).T.astype(np.float32).copy()
    nb2c[n2 - 5 * 128, 5] = 1.0   # lane 760 -> partition 120, block 5
    tiny32 = np.concatenate([b1c, nb2c], axis=1)

    shared = {"g_mat": g_sb, "w2t": w2_sb, "tiny32": tiny32}
    B = d.shape[0]
    bl = B // N_CORES
    in_maps = []
    for i in range(N_CORES):
        dT = d[i * bl:(i + 1) * bl, :].T.astype(np.float16)
        t16 = np.concatenate([dT, W1.astype(np.float16)], axis=1).copy()
        in_maps.append({**shared, "tiny16": t16})
    return in_maps


_nc_cache = {}


def kernel(d, W1, b1, W2, b2, A, b_eq):
    d = np.asarray(d, np.float32)
    W1 = np.asarray(W1, np.float32)
    b1 = np.asarray(b1, np.float32)
    W2 = np.asarray(W2, np.float32)
    b2 = np.asarray(b2, np.float32)
    A = np.asarray(A, np.float32)
    b_eq = np.asarray(b_eq, np.float32)

    if "nc" not in _nc_cache:
        _nc_cache["nc"] = _build()
    nc = _nc_cache["nc"]

    in_maps = _host_prepare(d, W1, b1, W2, b2, A, b_eq)
    res = run_bass_kernel_spmd(nc, in_maps, list(range(N_CORES)))

    outs = []
    for r in res.results:
        y = (r["y_out"].reshape(128, JT, BL).transpose(2, 1, 0)
             .reshape(BL, JT * 128))
        outs.append(y[:, :N2])
    return np.concatenate(outs, axis=0).astype(np.float32)


# revision 22
# speedup vs baseline: 1.1993x; 1.0026x over previous
"""nn_Cvx_ShortestPathNet — TRN2 Bass kernel, 8-core pure data parallelism.

Math (Dykstra alternating projections, c folded into G via a bias lane):
    G = A' pinv(AA') A  (projector),  c = b' pinv(AA') A
    Ghat[768,768]: Ghat[:760,:760] = G, Ghat[760,:760] = -c, Ghat[760,760] = 1
    negw lane 760 := 1 (via the b2 bias constant), so t lane 760 stays 1 and
    t@Ghat == t@G - c on real lanes.
    t_1 = negw = -MLP(d);  t_{k+1} = max(negw, t_k @ Ghat)   (pure tensor max)
    y = max(negw - t_K @ Ghat, 0) = max(ps, negw) - ps

On-chip layout transposed ([768, B_local], 6x128 partition tiles), B_local =
32 per core. Per iteration: 36 fp16 matmuls (f=32 moving rows -> 25ns issue
pitch, 900ns/iter floor) + 4 DVE tensor_tensor max ops (PARTITION: two
singles + two pairs), scheduled (staggered EDF order) so each op's result is
ready close to when the next iteration's matmuls consume it. Only DVE and
Act can read PSUM; Act's ~285ns ACTIVATE is slower than DVE's 190/225ns
ops, so everything element-wise stays on DVE (and the Act engine then never
stalls its own HWDGE DMA queue — engine activity pauses that engine's
hardware queue).

PSUM bank plan: start=True clears has_written for the WHOLE bank, so two
accumulation groups may not interleave within a bank -> private banks per
group. A pair op's tile is [128,1024] fp32 = TWO banks with the groups at
the bank edges (cols 480:512 | 512:544) so its DVE max reads one contiguous
[128,64]. Ops 0,1 (consumed right at iteration start) double-buffer across
iterations via p2pool (no write-after-read wait on their banks); warm-up and
MLP matmuls use scratch columns of the fixed pair tiles (clear of 480:544).
2x2 (pairs) + 2x2 (single bufs) = all 8 banks.

Startup: each dma_start issue costs ~650ns on its engine queue and the HWDGE
ring is ~4 deep, so inputs ride NINE dmas: packed tiny tensors, three 2-chunk
W2 dmas (they gate the MLP -> negw -> t_1 chain), three 2-chunk G dmas
ordered by iteration-1 first consumption ((0,1) then (2,3) then (4,5)).
W2 is negated on the host so negw = pw + nb2c needs no Act scale=-1 pass.
Dummy warm-up matmuls keep the PE busy through the DMA phase for the HAM
clock; iteration 1 chases the G chunk arrivals.

Batch 256 sharded 32 rows per core; Ghat, MLP weights replicated.
"""

import json
import numpy as np

import concourse.bass as bass
import concourse.mybir as mybir
import concourse.tile as tile
from concourse.bass_utils import run_bass_kernel_spmd

F32 = mybir.dt.float32
F16 = mybir.dt.float16
AT = mybir.AluOpType
AF = mybir.ActivationFunctionType

JT = 6          # 768/128 edge-dim tiles
BL = 32         # batch rows per core
HT = 5          # 640/128 hidden tiles
K_ITERS = 100
N_CORES = 8
N2 = 760
NP = JT * 128
PCOL = 480      # op tile: groups end at the bank edge (480:512 | 512:544)

# DVE op partition: groups covered by each tensor_tensor max op. Singles own
# one PSUM bank; pairs own two adjacent banks (group at each bank edge).
PARTITION = [(0,), (1,), (2, 3), (4, 5)]
# Staggered (j,k) order (EDF): group closes 17/19/21/23/29/35.
MM_ORDER = [
    (0, 1), (0, 0), (1, 1), (1, 0), (0, 2), (0, 3), (1, 2), (1, 3), (2, 2),
    (2, 3), (2, 1), (2, 0), (3, 2), (3, 3), (3, 1), (3, 0), (0, 4), (0, 5),
    (1, 4), (1, 5), (2, 4), (2, 5), (3, 4), (3, 5), (4, 4), (4, 5), (4, 2),
    (4, 3), (4, 1), (4, 0), (5, 4), (5, 5), (5, 2), (5, 3), (5, 1), (5, 0),
]
L2_ORDER = (4, 5, 2, 3, 0, 1)   # MLP layer-2 j order == W2 chunk arrivals
_FIRST = {}
_LAST = {}
for _pos, (_j, _k) in enumerate(MM_ORDER):
    _FIRST.setdefault(_j, _pos)
    _LAST[_j] = _pos
_OP_OF_GROUP = {}
_IDX_IN_OP = {}
for _oi, _S in enumerate(PARTITION):
    for _ix, _g in enumerate(_S):
        _OP_OF_GROUP[_g] = _oi
        _IDX_IN_OP[_g] = _ix
P2_OPS = (0, 1)                  # double-buffered (single-group) ops


def nw_base(oi):
    return sum(len(S) for S in PARTITION[:oi]) * BL


# ---------------------------------------------------------------------------
# This container's walrus build rejects instructions carrying more than one
# sync-wait. Split any multi-wait instruction at the BIR-JSON level: insert
# same-engine NoOps before it, each carrying one of the extra waits (waits
# are sem-ge, so order is irrelevant).
_orig_to_json_bytes = bass.Bass.to_json_bytes
_ctr = [0]


def _order_waits(engine: str, waits: list) -> list:
    """NoOps take the waits that are (almost surely) already satisfied --
    same-engine sems and DMA arrivals -- so the instruction keeps the
    latest-firing cross-engine wait and pays no NoOp decode after it."""
    def prio(w):
        nm = w.get("ant_name", "")
        if nm.startswith(engine + "_"):
            return 0
        if nm.startswith("DMA"):
            return 1
        if nm.startswith("PE_"):
            return 3
        return 2
    return sorted(waits, key=prio)


def _split_waits_json(raw: bytes) -> bytes:
    j = json.loads(raw)
    changed = False
    for fn in j.get("functions", []):
        for bb in fn.get("blocks", []):
            out = []
            for inst in bb.get("instructions", []):
                si = inst.get("sync_info") or {}
                waits = si.get("on_wait") or []
                if len(waits) > 1:
                    changed = True
                    waits = _order_waits(inst.get("engine", ""), waits)
                    for w in waits[:-1]:
                        _ctr[0] += 1
                        out.append({
                            "debug": inst.get("debug", 0),
                            "engine": inst["engine"],
                            "ins": [], "outs": [],
                            "name": f"I-waitsplit-{_ctr[0]}",
                            "opcode": "NoOp",
                            "sync_info": {"on_wait": [w], "on_update": []},
                        })
                    si["on_wait"] = waits[-1:]
                out.append(inst)
            bb["instructions"] = out
    return json.dumps(j).encode() if changed else raw


def _patched_to_json_bytes(self, *a, **k):
    return _split_waits_json(_orig_to_json_bytes(self, *a, **k))


bass.Bass.to_json_bytes = _patched_to_json_bytes


def _build(k_iters=K_ITERS):
    nc = bass.Bass("TRN2", target_bir_lowering=False, debug=False,
                   num_devices=N_CORES)

    g_mat = nc.dram_tensor("g_mat", [128, JT * JT * 128], F16, kind="ExternalInput").ap()
    w2t = nc.dram_tensor("w2t", [128, HT * JT * 128], F16, kind="ExternalInput").ap()
    tiny16 = nc.dram_tensor("tiny16", [64, BL + HT * 128], F16, kind="ExternalInput").ap()
    tiny32 = nc.dram_tensor("tiny32", [128, HT + JT], F32, kind="ExternalInput").ap()
    y_out = nc.dram_tensor("y_out", [128, JT * BL], F16, kind="ExternalOutput").ap()

    NOP = len(PARTITION)

    with tile.TileContext(nc) as tc:
        with (
            tc.tile_pool(name="const", bufs=1) as cpool,
            tc.tile_pool(name="state", bufs=2) as spool,
            tc.tile_pool(name="psum", bufs=1, space="PSUM") as ppool,
            tc.tile_pool(name="psum2", bufs=2, space="PSUM") as p2pool,
        ):
            # --- input DMAs ------------------------------------------------
            t16_sb = cpool.tile([64, BL + HT * 128], F16)
            dT_sb = t16_sb[:, 0:BL]
            w1_sb = t16_sb[:, BL:BL + HT * 128]
            t32_sb = cpool.tile([128, HT + JT], F32)
            b1c_sb = t32_sb[:, 0:HT]
            nb2c_sb = t32_sb[:, HT:HT + JT]
            w2_sb = cpool.tile([128, HT * JT * 128], F16)
            G_sb = cpool.tile([128, JT * JT * 128], F16)

            def w2sl(j2):
                return slice(j2 * 2 * HT * 128, (j2 + 1) * 2 * HT * 128)

            def gsl(k2):
                return slice(k2 * 2 * JT * 128, (k2 + 1) * 2 * JT * 128)

            # All DMAs issue up front (mid-program dma emission sprinkles
            # satisfied waits over downstream instructions, +5ns per matmul).
            # Empirically a consumer of DMA #k on a queue is released only
            # when DMA #k+1 on that queue completes, so each early-needed
            # DMA is chased by a 1KB dummy; the G chunks sit last per queue.
            dmy = [cpool.tile([128, 1], F32, name=f"dmy{i}") for i in range(4)]

            nc.sync.dma_start(out=t16_sb[:], in_=tiny16[:])
            nc.sync.dma_start(out=t32_sb[:], in_=tiny32[:])
            nc.sync.dma_start(out=dmy[0][:], in_=tiny32[:, 0:1])
            nc.sync.dma_start(out=w2_sb[:, w2sl(1)], in_=w2t[:, w2sl(1)])
            nc.sync.dma_start(out=dmy[1][:], in_=tiny32[:, 1:2])
            nc.sync.dma_start(out=G_sb[:, gsl(2)], in_=g_mat[:, gsl(2)])

            nc.scalar.dma_start(out=w2_sb[:, w2sl(2)], in_=w2t[:, w2sl(2)])
            nc.scalar.dma_start(out=dmy[2][:], in_=tiny32[:, 2:3])
            nc.scalar.dma_start(out=G_sb[:, gsl(0)], in_=g_mat[:, gsl(0)])

            nc.gpsimd.dma_start(out=w2_sb[:, w2sl(0)], in_=w2t[:, w2sl(0)])
            nc.gpsimd.dma_start(out=dmy[3][:], in_=tiny32[:, 3:4])
            nc.gpsimd.dma_start(out=G_sb[:, gsl(1)], in_=g_mat[:, gsl(1)])

            # fixed pair PSUM tiles for ops 2,3 (two banks each, groups at
            # the bank edges). Ops 0,1 get p2pool double buffers later.
            ps_fixed = {oi: ppool.tile([128, 1024], F32, tag=f"ps{oi}",
                                       name=f"psp{oi}")
                        for oi in range(NOP) if oi not in P2_OPS}
            # scratch for warm-up / MLP: four DISTINCT single-bank p2pool
            # tiles (tile-level WAR tracking would otherwise serialize the
            # whole MLP through two shared tiles). All scratch use completes
            # before iterations begin; iteration 0 then recycles the banks.
            scr = [p2pool.tile([128, 512], F32, tag=f"psg{P2_OPS[i % 2]}",
                               name=f"scr{i}") for i in range(4)]
            scratch = [t[:, 0:BL] for t in scr]
            _wctr = [0]

            def warm(n):
                # HAM warm-up: dummy matmuls keep the PE busy through the
                # DMA phase so the clock gate reaches K=8/8 before the
                # real work.
                for _ in range(n):
                    _wctr[0] += 1
                    nc.tensor.matmul(out=scratch[_wctr[0] % 4][:32, :],
                                     lhsT=dT_sb[:, :BL],
                                     rhs=dT_sb[:], start=True, stop=True)

            warm(12)

            # --- MLP (all element-wise work on DVE; Act engine stays idle
            # so its HWDGE queue streams undisturbed) -----------------------
            # h = leaky_relu(d@W1 + b1);  pw = -h@W2 (W2 negated on host);
            # negw = pw + nb2c  (nb2c = -b2, +1 on the bias lane)
            h16 = cpool.tile([128, HT * BL], F16)
            for m in range(HT):
                ph = scratch[m % 4]
                nc.tensor.matmul(out=ph[:, :],
                                 lhsT=w1_sb[:, m * 128:(m + 1) * 128],
                                 rhs=dT_sb[:], start=True, stop=True)
                pre = spool.tile([128, BL], F32, tag="pre", name=f"pre{m}")
                nc.vector.tensor_scalar(out=pre[:, :], in0=ph[:, :],
                                        scalar1=b1c_sb[:, m:m + 1],
                                        scalar2=None, op0=AT.add)
                # leaky relu = max(x, 0.1x)
                nc.vector.scalar_tensor_tensor(
                    out=h16[:, m * BL:(m + 1) * BL], in0=pre[:],
                    scalar=0.1, in1=pre[:], op0=AT.mult, op1=AT.max)
            warm(40)

            negw = [cpool.tile([128, len(S) * BL], F32, name=f"negw{oi}")
                    for oi, S in enumerate(PARTITION)]
            negw16 = [cpool.tile([128, len(S) * BL], F16, name=f"negw16_{oi}")
                      for oi, S in enumerate(PARTITION)]
            for jn, j in enumerate(L2_ORDER):
                pw = scratch[(j + 1) % 4]
                for k2 in range(HT):
                    nc.tensor.matmul(
                        out=pw[:, :],
                        lhsT=w2_sb[:, (j * HT + k2) * 128:(j * HT + k2 + 1) * 128],
                        rhs=h16[:, k2 * BL:(k2 + 1) * BL],
                        start=(k2 == 0), stop=(k2 == HT - 1))
                oi, lc = _OP_OF_GROUP[j], _IDX_IN_OP[j] * BL
                nc.vector.tensor_scalar(out=negw[oi][:, lc:lc + BL],
                                        in0=pw[:, :],
                                        scalar1=nb2c_sb[:, j:j + 1],
                                        scalar2=None, op0=AT.add)
                nc.vector.tensor_copy(out=negw16[oi][:, lc:lc + BL],
                                      in_=negw[oi][:, lc:lc + BL])
                if jn % 2 == 1 and jn < JT - 1:
                    warm(10)
            # fill the PE pipe while the G wave streams in (iteration 1
            # chases the G chunk arrivals right after); the HAM clock needs
            # a ~2.5us unbroken burst here to latch the full gate count
            warm(100)

            # t_1 = negw16 op tiles directly (read-only as rhs)
            t_cur = negw16

            def rhs_of(k, tiles):
                oi = _OP_OF_GROUP[k]
                lc = _IDX_IN_OP[k] * BL
                return tiles[oi][:, lc:lc + BL]

            # --- Dykstra iterations ---------------------------------------
            for it in range(k_iters):
                last_iter = it == k_iters - 1
                ps_it = {}
                for oi in range(NOP):
                    if oi in P2_OPS:
                        ps_it[oi] = p2pool.tile([128, 512], F32,
                                                tag=f"psg{oi}",
                                                name=f"psg{oi}_{it}")
                    else:
                        ps_it[oi] = ps_fixed[oi]

                def out_ap(j):
                    oi = _OP_OF_GROUP[j]
                    c = PCOL + _IDX_IN_OP[j] * BL
                    return ps_it[oi][:, c:c + BL]

                if not last_iter:
                    t_nxt = [spool.tile([128, len(S) * BL], F16,
                                        tag=f"t{oi}", name=f"t{it + 1}_{oi}")
                             for oi, S in enumerate(PARTITION)]
                else:
                    tlf = cpool.tile([128, JT * BL], F32)
                    y_sb = cpool.tile([128, JT * BL], F16)

                left = {j: 6 for j in range(JT)}
                for pos, (j, k) in enumerate(MM_ORDER):
                    nc.tensor.matmul(
                        out=out_ap(j),
                        lhsT=G_sb[:, (k * JT + j) * 128:(k * JT + j + 1) * 128],
                        rhs=rhs_of(k, t_cur),
                        start=(pos == _FIRST[j]), stop=(pos == _LAST[j]))
                    left[j] -= 1
                    oi = _OP_OF_GROUP[j]
                    if all(left[g] == 0 for g in PARTITION[oi]):
                        for g in PARTITION[oi]:
                            left[g] = -1  # fire once
                        n = len(PARTITION[oi]) * BL
                        pss = ps_it[oi][:, PCOL:PCOL + n]
                        if not last_iter:
                            nc.vector.tensor_tensor(out=t_nxt[oi][:],
                                                    in0=pss,
                                                    in1=negw16[oi][:],
                                                    op=AT.max)
                        else:
                            gl = slice(nw_base(oi), nw_base(oi) + n)
                            nc.vector.tensor_tensor(out=tlf[:, gl], in0=pss,
                                                    in1=negw[oi][:],
                                                    op=AT.max)
                            nc.vector.tensor_tensor(out=y_sb[:, gl],
                                                    in0=tlf[:, gl], in1=pss,
                                                    op=AT.subtract)
                if not last_iter:
                    t_cur = t_nxt
                else:
                    # output DMAs fire as op results complete (ops 0+1 /
                    # op 2 / op 3), one per queue
                    nc.sync.dma_start(out=y_out[:, 0:2 * BL],
                                      in_=y_sb[:, 0:2 * BL])
                    nc.scalar.dma_start(out=y_out[:, 2 * BL:4 * BL],
                                        in_=y_sb[:, 2 * BL:4 * BL])
                    nc.gpsimd.dma_start(out=y_out[:, 4 * BL:6 * BL],
                                        in_=y_sb[:, 4 * BL:6 * BL])
    return nc


def _host_prepare(d, W1, b1, W2, b2, A, b_eq):
    A64 = A.astype(np.float64)
    M = np.linalg.pinv(A64 @ A64.T)
    G = A64.T @ M @ A64
    c = (b_eq.astype(np.float64) @ M) @ A64

    n2 = A.shape[1]
    Ghat = np.zeros((NP, NP), np.float64)
    Ghat[:n2, :n2] = G
    Ghat[n2, :n2] = -c          # bias lane row
    Ghat[n2, n2] = 1.0

    g_sb = (Ghat.reshape(JT, 128, JT, 128).transpose(1, 0, 2, 3)
            .reshape(128, JT * JT * 128)).astype(np.float16)

    HID = W1.shape[1]
    W2_pad = np.zeros((HID, NP), np.float64)
    W2_pad[:, :n2] = -W2.astype(np.float64)   # negated: negw = pw + nb2c
    w2_sb = (W2_pad.reshape(HT, 128, JT, 128).transpose(1, 2, 0, 3)
             .reshape(128, JT * HT * 128)).astype(np.float16)
    b1c = b1.reshape(HT, 128).T.astype(np.float32).copy()
    b2_pad = np.zeros(NP, np.float32)
    b2_pad[:n2] = b2
    nb2c = (-b2_pad).reshape(JT, 128).T.astype(np.float32).copy()
    nb2c[n2 - 5 * 128, 5] = 1.0   # lane 760 -> partition 120, block 5
    tiny32 = np.concatenate([b1c, nb2c], axis=1)

    shared = {"g_mat": g_sb, "w2t": w2_sb, "tiny32": tiny32}
    B = d.shape[0]
    bl = B // N_CORES
    in_maps = []
    for i in range(N_CORES):
        dT = d[i * bl:(i + 1) * bl, :].T.astype(np.float16)
        t16 = np.concatenate([dT, W1.astype(np.float16)], axis=1).copy()
        in_maps.append({**shared, "tiny16": t16})
    return in_maps


_nc_cache = {}


def kernel(d, W1, b1, W2, b2, A, b_eq):
    d = np.asarray(d, np.float32)
    W1 = np.asarray(W1, np.float32)
    b1 = np.asarray(b1, np.float32)
    W2 = np.asarray(W2, np.float32)
    b2 = np.asarray(b2, np.float32)
    A = np.asarray(A, np.float32)
    b_eq = np.asarray(b_eq, np.float32)

    if "nc" not in _nc_cache:
        _nc_cache["nc"] = _build()
    nc = _nc_cache["nc"]

    in_maps = _host_prepare(d, W1, b1, W2, b2, A, b_eq)
    res = run_bass_kernel_spmd(nc, in_maps, list(range(N_CORES)))

    outs = []
    for r in res.results:
        y = (r["y_out"].reshape(128, JT, BL).transpose(2, 1, 0)
             .reshape(BL, JT * 128))
        outs.append(y[:, :N2])
    return np.concatenate(outs, axis=0).astype(np.float32)
